# revision 12
# baseline (speedup 1.0000x reference)
"""Farthest Point Sampling (FPS) Bass/TRN2 kernel.

Problem: pos [16, 16384, 3] f32 -> indices [16*2048] int32 (exact FPS,
start index 0, ratio 1/8), bit-exact trajectory vs the f32 reference.

Sharding: batch 16 clouds -> 8 NeuronCores, 2 clouds per core (data
parallel, no cross-core communication). Each cloud is laid out as
[128 partitions, 128 free] (point n -> (n//128, n%128)).

Per FPS step per cloud (raw bass, manual semaphores):
  ACT : SQX/SQY/SQZ = Square(coord + bias)       bias = -c (per-partition AP)
  DVE : t1 = SQX+SQY; d = t1+SQZ; DIST = min(DIST, d); rowmax = max_f(DIST)
  PE  : rmT = rowmax^T                           (matmul vs identity -> PSUM)
  DVE : M = max(rmT)                             [1,1]
  PE  : Mb = ones_row^T @ M                      broadcast M -> [128,1] PSUM
  DVE : MASK = is_equal(DIST, Mb)                single-hot (no ties, verified)
  DVE : SCR4 = MEGA4 * MASK(x4);  RS = reduce_add -> [128,4]  (MEGA4 = [X|Y|Z|GIOTA])
  PE  : NEGBC = (-1)^T128 @ RS -> PSUM [128,4]   = (-cx,-cy,-cz,-n*) everywhere
  ACT : BIAS = NEGBC[:,0:3] -> SBUF;  OUTROW[0, 4s:4s+4] = NEGBC[0,:]
Host decodes n* = -OUTROW[4s+3]."""

import numpy as np
from contextlib import ExitStack

import concourse.bass as bass
import concourse.mybir as mybir
from concourse.bass_utils import run_bass_kernel_spmd

AT = mybir.ActivationFunctionType
AL = mybir.AluOpType
AX = mybir.AxisListType
F32 = mybir.dt.float32

B, N, S = 16, 16384, 2048
N_CORES = 8
N_CLOUDS = 2  # per core
BIG = 1.0e10

_CACHE = {}
LABELS = {}


def _build_fps_kernel(S=S, n_clouds=N_CLOUDS):
    nc = bass.Bass(trn_type="TRN2", detect_race_conditions=False)
    mega_d = nc.dram_tensor("mega", [n_clouds, 128, 512], F32, kind="ExternalInput")
    bias0_d = nc.dram_tensor("bias0", [n_clouds, 128, 3], F32, kind="ExternalInput")
    ident_d = nc.dram_tensor("ident", [128, 128], F32, kind="ExternalInput")
    onesrow_d = nc.dram_tensor("onesrow", [1, 128], F32, kind="ExternalInput")
    negones_d = nc.dram_tensor("negones", [128, 128], F32, kind="ExternalInput")
    out_d = nc.dram_tensor("outrow", [n_clouds, 4 * S], F32, kind="ExternalOutput")

    es = ExitStack()
    counter = [0]

    def sb(shape):
        counter[0] += 1
        return es.enter_context(nc.sbuf_tensor(f"sb{counter[0]}", shape, F32))

    def ps(shape):
        counter[0] += 1
        return es.enter_context(nc.psum_tensor(f"ps{counter[0]}", shape, F32))

    ident = sb([128, 128])
    onesrow = sb([1, 128])
    negones = sb([128, 128])

    cl = []
    for c in range(n_clouds):
        cl.append(dict(
            mega=sb([128, 512]),
            dist=sb([128, 128]),
            sqx=sb([128, 128]), sqy=sb([128, 128]), sqz=sb([128, 128]),
            t1=sb([128, 128]), dd=sb([128, 128]),
            mask=sb([128, 128]),
            scr4=sb([128, 512]),
            rs=sb([128, 4]),
            rowmax=sb([128, 1]),
            msb=sb([1, 1]),
            biassb=sb([128, 3]),
            outrow=sb([1, 4 * S]),
            rmt_ps=ps([1, 128]),
            mb_ps=ps([128, 1]),
            negbc_ps=ps([128, 4]),
        ))

    sem_act = es.enter_context(nc.semaphore())
    sem_dve = es.enter_context(nc.semaphore())
    sem_pe = es.enter_context(nc.semaphore())
    sem_gp = es.enter_context(nc.semaphore())

    sems = {"act": sem_act, "dve": sem_dve, "pe": sem_pe, "gp": sem_gp}
    engines = {"act": nc.scalar, "dve": nc.vector, "pe": nc.tensor, "gp": nc.gpsimd}
    count = {k: 0 for k in sems}
    waited = {(a, b): 0 for a in sems for b in sems}
    label = [None]

    def emit(eng, instr, inc=1):
        instr.then_inc(sems[eng], inc)
        count[eng] += inc
        if label[0] is not None:
            try:
                LABELS[instr.ins.name] = label[0]
            except Exception:
                pass
        return count[eng]

    def wait(consumer, producer, tick):
        if tick is None or consumer == producer:
            return
        if waited[(consumer, producer)] < tick:
            engines[consumer].wait_ge(sems[producer], tick)
            waited[(consumer, producer)] = tick

    for c in range(n_clouds):
        emit("gp", nc.gpsimd.dma_start(cl[c]["mega"][:], mega_d[c]), 16)
        emit("gp", nc.gpsimd.dma_start(cl[c]["biassb"][:], bias0_d[c]), 16)
    emit("gp", nc.gpsimd.dma_start(ident[:], ident_d[:]), 16)
    emit("gp", nc.gpsimd.dma_start(onesrow[:], onesrow_d[:]), 16)
    emit("gp", nc.gpsimd.dma_start(negones[:], negones_d[:]), 16)
    dma0 = count["gp"]
    for c in range(n_clouds):
        wait("dve", "gp", dma0)
        emit("dve", nc.vector.memset(cl[c]["dist"][:], BIG))
        emit("dve", nc.vector.memset(cl[c]["outrow"][:], 0.0))
    wait("act", "gp", dma0)
    wait("pe", "gp", dma0)

    ticks = [dict() for _ in range(n_clouds)]

    def phase_update(c, s):
        t, tk = cl[c], ticks[c]
        label[0] = f"{'AB'[c]}.upd"
        for j, sq in enumerate(("sqx", "sqy", "sqz")):
            tk[sq] = emit("act", nc.scalar.activation(
                t[sq][:], t["mega"][:, j * 128:(j + 1) * 128], AT.Square,
                bias=t["biassb"][:, j:j + 1], scale=1.0))
        wait("dve", "act", tk["sqy"])
        tk["t1"] = emit("dve", nc.vector.tensor_tensor(t["t1"][:], t["sqx"][:], t["sqy"][:], AL.add))
        wait("dve", "act", tk["sqz"])
        tk["d"] = emit("dve", nc.vector.tensor_tensor(t["dd"][:], t["t1"][:], t["sqz"][:], AL.add))
        tk["min"] = emit("dve", nc.vector.tensor_tensor(t["dist"][:], t["dist"][:], t["dd"][:], AL.min))
        tk["rowmax"] = emit("dve", nc.vector.reduce_max(t["rowmax"][:, 0:1], t["dist"][:], axis=AX.X))

    def phase_argmax(c, s):
        t, tk = cl[c], ticks[c]
        label[0] = f"{'AB'[c]}.arg"
        wait("pe", "dve", tk["rowmax"])
        tk["rmt"] = emit("pe", nc.tensor.matmul(t["rmt_ps"][:], t["rowmax"][:, 0:1], ident[:], start=True, stop=True))
        wait("dve", "pe", tk["rmt"])
        tk["m"] = emit("dve", nc.vector.reduce_max(t["msb"][0:1, 0:1], t["rmt_ps"][0:1, :], axis=AX.X))
        wait("pe", "dve", tk["m"])
        tk["mb"] = emit("pe", nc.tensor.matmul(t["mb_ps"][:], onesrow[0:1, :], t["msb"][0:1, 0:1], start=True, stop=True))
        wait("dve", "pe", tk["mb"])
        tk["mask"] = emit("dve", nc.vector.tensor_tensor(
            t["mask"][:], t["dist"][:], t["mb_ps"][:, 0:1].broadcast_to((128, 128)), AL.is_equal))

    def phase_gather(c, s):
        t, tk = cl[c], ticks[c]
        label[0] = f"{'AB'[c]}.gat"
        mask_rep = t["mask"][:].rearrange("p (a f) -> p a f", a=1).broadcast_to((128, 4, 128))
        tk["mul"] = emit("dve", nc.vector.tensor_tensor(t["scr4"][:], t["mega"][:], mask_rep, AL.mult))
        tk["rs"] = emit("dve", nc.vector.tensor_reduce(
            t["rs"][:, 0:4], t["scr4"][:].rearrange("p (k f) -> p k f", k=4), axis=AX.X, op=AL.add))

    def phase_tail(c, s):
        t, tk = cl[c], ticks[c]
        label[0] = f"{'AB'[c]}.tai"
        wait("pe", "dve", tk["rs"])
        tk["negbc"] = emit("pe", nc.tensor.matmul(t["negbc_ps"][:], negones[:], t["rs"][:, 0:4], start=True, stop=True))
        wait("act", "pe", tk["negbc"])
        tk["bias"] = emit("act", nc.scalar.copy(t["biassb"][:], t["negbc_ps"][:, 0:3]))
        tk["out"] = emit("act", nc.scalar.copy(t["outrow"][0:1, 4 * s:4 * s + 4], t["negbc_ps"][0:1, 0:4]))

    if n_clouds == 2:
        # software-pipelined: cloud 1 runs ~half a step behind cloud 0 so the
        # two serial chains interleave on the engines instead of running
        # back-to-back.
        A, Bc = 0, 1
        phase_update(A, 1)
        phase_argmax(A, 1)
        phase_update(Bc, 1)
        phase_gather(A, 1)
        phase_argmax(Bc, 1)
        phase_tail(A, 1)
        phase_gather(Bc, 1)
        for s in range(2, S):
            phase_update(A, s)
            phase_tail(Bc, s - 1)
            phase_update(Bc, s)
            phase_argmax(A, s)
            phase_gather(A, s)
            phase_tail(A, s)
            phase_argmax(Bc, s)
            phase_gather(Bc, s)
        phase_tail(Bc, S - 1)
    else:
        for s in range(1, S):
            for c in range(n_clouds):
                phase_update(c, s)
            for c in range(n_clouds):
                phase_argmax(c, s)
            for c in range(n_clouds):
                phase_gather(c, s)
            for c in range(n_clouds):
                phase_tail(c, s)

    for c in range(n_clouds):
        wait("gp", "act", ticks[c]["out"])
        emit("gp", nc.gpsimd.dma_start(out_d[c], cl[c]["outrow"][0:1, :]), 16)

    es.close()
    return nc


def _make_inputs(pos_pair):
    ncl = pos_pair.shape[0]
    mega = np.empty((ncl, 128, 512), np.float32)
    bias0 = np.empty((ncl, 128, 3), np.float32)
    gi = np.arange(N, dtype=np.float32).reshape(128, 128)
    for c in range(ncl):
        for j in range(3):
            mega[c, :, j * 128:(j + 1) * 128] = pos_pair[c, :, j].reshape(128, 128)
        mega[c, :, 384:512] = gi
        bias0[c] = -pos_pair[c, 0]
    return {
        "mega": mega,
        "bias0": bias0,
        "ident": np.eye(128, dtype=np.float32),
        "onesrow": np.ones((1, 128), np.float32),
        "negones": np.full((128, 128), -1.0, np.float32),
    }


def _get_nc():
    if "nc" not in _CACHE:
        _CACHE["nc"] = _build_fps_kernel()
    return _CACHE["nc"]


def run_on_cores(pos, **spmd_kwargs):
    """pos [16, 16384, 3] f32 -> (idx [16*2048] int32, BassKernelResults)."""
    pos = np.ascontiguousarray(np.asarray(pos, dtype=np.float32))
    assert pos.shape == (B, N, 3)
    nc = _get_nc()
    in_maps = [_make_inputs(pos[N_CLOUDS * c: N_CLOUDS * (c + 1)]) for c in range(N_CORES)]
    res = run_bass_kernel_spmd(nc, in_maps, core_ids=list(range(N_CORES)), **spmd_kwargs)
    idx = np.empty((B, S), np.int32)
    for core in range(N_CORES):
        outrow = res.results[core]["outrow"]  # [n_clouds, 4S]
        loc = np.rint(-outrow[:, 3::4]).astype(np.int32)
        loc[:, 0] = 0
        for c in range(N_CLOUDS):
            b = N_CLOUDS * core + c
            idx[b] = loc[c] + b * N
    return idx.reshape(-1), res


def kernel(pos):
    idx, _ = run_on_cores(pos)
    return idx


# revision 13
# speedup vs baseline: 1.0359x; 1.0359x over previous
"""Farthest Point Sampling (FPS) Bass/TRN2 kernel.

Problem: pos [16, 16384, 3] f32 -> indices [16*2048] int32 (exact FPS,
start index 0, ratio 1/8), bit-exact trajectory vs the f32 reference.

Sharding: batch 16 clouds -> 8 NeuronCores, 2 clouds per core (data
parallel, no cross-core communication). Each cloud is laid out as
[128 partitions, 128 free] (point n -> (n//128, n%128)).

Per FPS step per cloud (raw bass, manual semaphores):
  ACT : SQX/SQY/SQZ = Square(coord + bias)       bias = -c (per-partition AP)
  DVE : t1 = SQX+SQY; d = t1+SQZ; DIST = min(DIST, d); rowmax = max_f(DIST)
  PE  : rmT = rowmax^T                           (matmul vs identity -> PSUM)
  DVE : M = max(rmT)                             [1,1]
  PE  : Mb = ones_row^T @ M                      broadcast M -> [128,1] PSUM
  DVE : MASK = is_equal(DIST, Mb)                single-hot (no ties, verified)
  DVE : SCR4 = MEGA4 * MASK(x4);  RS = reduce_add -> [128,4]  (MEGA4 = [X|Y|Z|GIOTA])
  PE  : NEGBC = (-1)^T128 @ RS -> PSUM [128,4]   = (-cx,-cy,-cz,-n*) everywhere
  ACT : BIAS = NEGBC[:,0:3] -> SBUF;  OUTROW[0, 4s:4s+4] = NEGBC[0,:]
Host decodes n* = -OUTROW[4s+3]."""

import numpy as np
from contextlib import ExitStack

import concourse.bass as bass
import concourse.mybir as mybir
from concourse.bass_utils import run_bass_kernel_spmd

AT = mybir.ActivationFunctionType
AL = mybir.AluOpType
AX = mybir.AxisListType
F32 = mybir.dt.float32

B, N, S = 16, 16384, 2048
N_CORES = 8
N_CLOUDS = 2  # per core
BIG = 1.0e10

_CACHE = {}
LABELS = {}


def _build_fps_kernel(S=S, n_clouds=N_CLOUDS):
    nc = bass.Bass(trn_type="TRN2", detect_race_conditions=False)
    mega_d = nc.dram_tensor("mega", [n_clouds, 128, 512], F32, kind="ExternalInput")
    bias0_d = nc.dram_tensor("bias0", [n_clouds, 128, 3], F32, kind="ExternalInput")
    ident_d = nc.dram_tensor("ident", [128, 128], F32, kind="ExternalInput")
    onesrow_d = nc.dram_tensor("onesrow", [1, 128], F32, kind="ExternalInput")
    negones_d = nc.dram_tensor("negones", [128, 128], F32, kind="ExternalInput")
    out_d = nc.dram_tensor("outrow", [n_clouds, 4 * S], F32, kind="ExternalOutput")

    es = ExitStack()
    counter = [0]

    def sb(shape):
        counter[0] += 1
        return es.enter_context(nc.sbuf_tensor(f"sb{counter[0]}", shape, F32))

    def ps(shape):
        counter[0] += 1
        return es.enter_context(nc.psum_tensor(f"ps{counter[0]}", shape, F32))

    ident = sb([128, 128])
    onesrow = sb([1, 128])
    negones = sb([128, 128])

    cl = []
    for c in range(n_clouds):
        cl.append(dict(
            mega=sb([128, 512]),
            dist=sb([128, 128]),
            sqx=sb([128, 128]), sqy=sb([128, 128]), sqz=sb([128, 128]),
            t1=sb([128, 128]), dd=sb([128, 128]),
            mask=sb([128, 128]),
            scr4=sb([128, 512]),
            rs=sb([128, 4]),
            rowmax=sb([128, 1]),
            msb=sb([1, 1]),
            biassb=sb([128, 3]),
            outrow=sb([1, 4 * S]),
            rmt_ps=ps([1, 128]),
            mb_ps=ps([128, 1]),
            negbc_ps=ps([128, 4]),
        ))

    sem_act = es.enter_context(nc.semaphore())
    sem_dve = es.enter_context(nc.semaphore())
    sem_pe = es.enter_context(nc.semaphore())
    sem_gp = es.enter_context(nc.semaphore())

    sems = {"act": sem_act, "dve": sem_dve, "pe": sem_pe, "gp": sem_gp}
    engines = {"act": nc.scalar, "dve": nc.vector, "pe": nc.tensor, "gp": nc.gpsimd}
    count = {k: 0 for k in sems}
    waited = {(a, b): 0 for a in sems for b in sems}
    label = [None]

    def emit(eng, instr, inc=1):
        instr.then_inc(sems[eng], inc)
        count[eng] += inc
        if label[0] is not None:
            try:
                LABELS[instr.ins.name] = label[0]
            except Exception:
                pass
        return count[eng]

    def wait(consumer, producer, tick):
        if tick is None or consumer == producer:
            return
        if waited[(consumer, producer)] < tick:
            engines[consumer].wait_ge(sems[producer], tick)
            waited[(consumer, producer)] = tick

    for c in range(n_clouds):
        emit("gp", nc.gpsimd.dma_start(cl[c]["mega"][:], mega_d[c]), 16)
        emit("gp", nc.gpsimd.dma_start(cl[c]["biassb"][:], bias0_d[c]), 16)
    emit("gp", nc.gpsimd.dma_start(ident[:], ident_d[:]), 16)
    emit("gp", nc.gpsimd.dma_start(onesrow[:], onesrow_d[:]), 16)
    emit("gp", nc.gpsimd.dma_start(negones[:], negones_d[:]), 16)
    dma0 = count["gp"]
    for c in range(n_clouds):
        wait("dve", "gp", dma0)
        emit("dve", nc.vector.memset(cl[c]["dist"][:], BIG))
        emit("dve", nc.vector.memset(cl[c]["outrow"][:], 0.0))
    wait("act", "gp", dma0)
    wait("pe", "gp", dma0)

    ticks = [dict() for _ in range(n_clouds)]

    def phase_update(c, s):
        t, tk = cl[c], ticks[c]
        label[0] = f"{'AB'[c]}.upd"
        for j, sq in enumerate(("sqx", "sqy", "sqz")):
            tk[sq] = emit("act", nc.scalar.activation(
                t[sq][:], t["mega"][:, j * 128:(j + 1) * 128], AT.Square,
                bias=t["biassb"][:, j:j + 1], scale=1.0))
        wait("dve", "act", tk["sqy"])
        tk["t1"] = emit("dve", nc.vector.tensor_tensor(t["t1"][:], t["sqx"][:], t["sqy"][:], AL.add))
        wait("dve", "act", tk["sqz"])
        tk["d"] = emit("dve", nc.vector.tensor_tensor(t["dd"][:], t["t1"][:], t["sqz"][:], AL.add))
        tk["min"] = emit("dve", nc.vector.tensor_tensor(t["dist"][:], t["dist"][:], t["dd"][:], AL.min))
        tk["rowmax"] = emit("dve", nc.vector.reduce_max(t["rowmax"][:, 0:1], t["dist"][:], axis=AX.X))

    def phase_argmax(c, s):
        t, tk = cl[c], ticks[c]
        label[0] = f"{'AB'[c]}.arg"
        wait("pe", "dve", tk["rowmax"])
        tk["rmt"] = emit("pe", nc.tensor.matmul(t["rmt_ps"][:], t["rowmax"][:, 0:1], ident[:], start=True, stop=True))
        wait("dve", "pe", tk["rmt"])
        tk["m"] = emit("dve", nc.vector.reduce_max(t["msb"][0:1, 0:1], t["rmt_ps"][0:1, :], axis=AX.X))
        wait("pe", "dve", tk["m"])
        tk["mb"] = emit("pe", nc.tensor.matmul(t["mb_ps"][:], onesrow[0:1, :], t["msb"][0:1, 0:1], start=True, stop=True))
        wait("dve", "pe", tk["mb"])
        tk["mask"] = emit("dve", nc.vector.tensor_tensor(
            t["mask"][:], t["dist"][:], t["mb_ps"][:, 0:1].broadcast_to((128, 128)), AL.is_equal))

    def phase_gather(c, s):
        t, tk = cl[c], ticks[c]
        label[0] = f"{'AB'[c]}.gat"
        mask_rep = t["mask"][:].rearrange("p (a f) -> p a f", a=1).broadcast_to((128, 4, 128))
        tk["mul"] = emit("dve", nc.vector.tensor_tensor(t["scr4"][:], t["mega"][:], mask_rep, AL.mult))
        tk["rs"] = emit("dve", nc.vector.tensor_reduce(
            t["rs"][:, 0:4], t["scr4"][:].rearrange("p (k f) -> p k f", k=4), axis=AX.X, op=AL.add))

    def phase_tail(c, s):
        t, tk = cl[c], ticks[c]
        label[0] = f"{'AB'[c]}.tai"
        wait("pe", "dve", tk["rs"])
        tk["negbc"] = emit("pe", nc.tensor.matmul(t["negbc_ps"][:], negones[:], t["rs"][:, 0:4], start=True, stop=True))
        wait("act", "pe", tk["negbc"])
        tk["bias"] = emit("act", nc.scalar.copy(t["biassb"][:], t["negbc_ps"][:, 0:3]))
        tk["out"] = emit("act", nc.scalar.copy(t["outrow"][0:1, 4 * s:4 * s + 4], t["negbc_ps"][0:1, 0:4]))

    if n_clouds == 2:
        # software-pipelined: cloud 1 runs ~half a step behind cloud 0 so the
        # two serial chains interleave on the engines instead of running
        # back-to-back.
        A, Bc = 0, 1
        phase_update(A, 1)
        phase_argmax(A, 1)
        phase_update(Bc, 1)
        phase_gather(A, 1)
        phase_argmax(Bc, 1)
        phase_tail(A, 1)
        phase_gather(Bc, 1)
        for s in range(2, S):
            phase_update(A, s)
            phase_tail(Bc, s - 1)
            phase_argmax(A, s)
            phase_update(Bc, s)
            phase_gather(A, s)
            phase_tail(A, s)
            phase_argmax(Bc, s)
            phase_gather(Bc, s)
        phase_tail(Bc, S - 1)
    else:
        for s in range(1, S):
            for c in range(n_clouds):
                phase_update(c, s)
            for c in range(n_clouds):
                phase_argmax(c, s)
            for c in range(n_clouds):
                phase_gather(c, s)
            for c in range(n_clouds):
                phase_tail(c, s)

    for c in range(n_clouds):
        wait("gp", "act", ticks[c]["out"])
        emit("gp", nc.gpsimd.dma_start(out_d[c], cl[c]["outrow"][0:1, :]), 16)

    es.close()
    return nc


def _make_inputs(pos_pair):
    ncl = pos_pair.shape[0]
    mega = np.empty((ncl, 128, 512), np.float32)
    bias0 = np.empty((ncl, 128, 3), np.float32)
    gi = np.arange(N, dtype=np.float32).reshape(128, 128)
    for c in range(ncl):
        for j in range(3):
            mega[c, :, j * 128:(j + 1) * 128] = pos_pair[c, :, j].reshape(128, 128)
        mega[c, :, 384:512] = gi
        bias0[c] = -pos_pair[c, 0]
    return {
        "mega": mega,
        "bias0": bias0,
        "ident": np.eye(128, dtype=np.float32),
        "onesrow": np.ones((1, 128), np.float32),
        "negones": np.full((128, 128), -1.0, np.float32),
    }


def _get_nc():
    if "nc" not in _CACHE:
        _CACHE["nc"] = _build_fps_kernel()
    return _CACHE["nc"]


def run_on_cores(pos, **spmd_kwargs):
    """pos [16, 16384, 3] f32 -> (idx [16*2048] int32, BassKernelResults)."""
    pos = np.ascontiguousarray(np.asarray(pos, dtype=np.float32))
    assert pos.shape == (B, N, 3)
    nc = _get_nc()
    in_maps = [_make_inputs(pos[N_CLOUDS * c: N_CLOUDS * (c + 1)]) for c in range(N_CORES)]
    res = run_bass_kernel_spmd(nc, in_maps, core_ids=list(range(N_CORES)), **spmd_kwargs)
    idx = np.empty((B, S), np.int32)
    for core in range(N_CORES):
        outrow = res.results[core]["outrow"]  # [n_clouds, 4S]
        loc = np.rint(-outrow[:, 3::4]).astype(np.int32)
        loc[:, 0] = 0
        for c in range(N_CLOUDS):
            b = N_CLOUDS * core + c
            idx[b] = loc[c] + b * N
    return idx.reshape(-1), res


def kernel(pos):
    idx, _ = run_on_cores(pos)
    return idx


# revision 15
# speedup vs baseline: 1.2467x; 1.2036x over previous
"""Farthest Point Sampling (FPS) Bass/TRN2 kernel.

Problem: pos [16, 16384, 3] f32 -> indices [16*2048] int32 (exact FPS,
start index 0, ratio 1/8), bit-exact trajectory vs the f32 reference.

Sharding: batch 16 clouds -> 8 NeuronCores, 2 clouds per core (data
parallel, no cross-core communication). Each cloud is laid out as
[128 partitions, 128 free] (point n -> (n//128, n%128)).

Per FPS step per cloud (raw bass, manual semaphores):
  ACT : SQX/SQY/SQZ = Square(coord + bias)       bias = -c (per-partition AP)
  DVE : t1 = SQX+SQY; d = t1+SQZ; DIST = min(DIST, d); rowmax = max_f(DIST)
  PE  : rmT = rowmax^T                           (matmul vs identity -> PSUM)
  DVE : M = max(rmT)                             [1,1]
  PE  : Mb = ones_row^T @ M                      broadcast M -> [128,1] PSUM
  DVE : MASK = is_equal(DIST, Mb)                single-hot (no ties, verified)
  DVE : SCR4 = MEGA4 * MASK(x4);  RS = reduce_add -> [128,4]  (MEGA4 = [X|Y|Z|GIOTA])
  PE  : NEGBC = (-1)^T128 @ RS -> PSUM [128,4]   = (-cx,-cy,-cz,-n*) everywhere
  ACT : BIAS = NEGBC[:,0:3] -> SBUF;  OUTROW[0, 4s:4s+4] = NEGBC[0,:]
Host decodes n* = -OUTROW[4s+3]."""

import numpy as np
from contextlib import ExitStack

import concourse.bass as bass
import concourse.mybir as mybir
from concourse.bass_utils import run_bass_kernel_spmd

AT = mybir.ActivationFunctionType
AL = mybir.AluOpType
AX = mybir.AxisListType
F32 = mybir.dt.float32

B, N, S = 16, 16384, 2048
N_CORES = 8
N_CLOUDS = 2  # per core
BIG = 1.0e10

_CACHE = {}
LABELS = {}


def _build_fps_kernel(S=S, n_clouds=N_CLOUDS):
    nc = bass.Bass(trn_type="TRN2", detect_race_conditions=False)
    mega_d = nc.dram_tensor("mega", [n_clouds, 128, 512], F32, kind="ExternalInput")
    bias0_d = nc.dram_tensor("bias0", [n_clouds, 128, 3], F32, kind="ExternalInput")
    ident_d = nc.dram_tensor("ident", [128, 128], F32, kind="ExternalInput")
    onesrow_d = nc.dram_tensor("onesrow", [1, 128], F32, kind="ExternalInput")
    negones_d = nc.dram_tensor("negones", [128, 128], F32, kind="ExternalInput")
    out_d = nc.dram_tensor("outrow", [n_clouds, 4 * S], F32, kind="ExternalOutput")

    es = ExitStack()
    counter = [0]

    def sb(shape):
        counter[0] += 1
        return es.enter_context(nc.sbuf_tensor(f"sb{counter[0]}", shape, F32))

    def ps(shape):
        counter[0] += 1
        return es.enter_context(nc.psum_tensor(f"ps{counter[0]}", shape, F32))

    ident = sb([128, 128])
    onesrow = sb([1, 128])
    negones = sb([128, 128])

    cl = []
    for c in range(n_clouds):
        cl.append(dict(
            mega=sb([128, 512]),
            dist=sb([128, 128]),
            sqx=sb([128, 128]), sqy=sb([128, 128]), sqz=sb([128, 128]),
            t1=sb([128, 128]), dd=sb([128, 128]),
            mask=sb([128, 128]),
            scr4=sb([128, 512]),
            rs=sb([128, 4]),
            rowmax=sb([128, 1]),
            msb=sb([1, 1]),
            biassb=sb([128, 3]),
            outrow=sb([1, 4 * S]),
            rmt_ps=ps([1, 128]),
            mb_ps=ps([128, 1]),
            negbc_ps=ps([128, 4]),
        ))

    sem_act = es.enter_context(nc.semaphore())
    sem_dve = es.enter_context(nc.semaphore())
    sem_pe = es.enter_context(nc.semaphore())
    sem_gp = es.enter_context(nc.semaphore())

    sems = {"act": sem_act, "dve": sem_dve, "pe": sem_pe, "gp": sem_gp}
    engines = {"act": nc.scalar, "dve": nc.vector, "pe": nc.tensor, "gp": nc.gpsimd}
    count = {k: 0 for k in sems}
    waited = {(a, b): 0 for a in sems for b in sems}
    label = [None]

    def emit(eng, instr, inc=1):
        instr.then_inc(sems[eng], inc)
        count[eng] += inc
        if label[0] is not None:
            try:
                LABELS[instr.ins.name] = label[0]
            except Exception:
                pass
        return count[eng]

    def wait(consumer, producer, tick):
        if tick is None or consumer == producer:
            return
        if waited[(consumer, producer)] < tick:
            engines[consumer].wait_ge(sems[producer], tick)
            waited[(consumer, producer)] = tick

    for c in range(n_clouds):
        emit("gp", nc.gpsimd.dma_start(cl[c]["mega"][:], mega_d[c]), 16)
        emit("gp", nc.gpsimd.dma_start(cl[c]["biassb"][:], bias0_d[c]), 16)
    emit("gp", nc.gpsimd.dma_start(ident[:], ident_d[:]), 16)
    emit("gp", nc.gpsimd.dma_start(onesrow[:], onesrow_d[:]), 16)
    emit("gp", nc.gpsimd.dma_start(negones[:], negones_d[:]), 16)
    dma0 = count["gp"]
    for c in range(n_clouds):
        wait("dve", "gp", dma0)
        emit("dve", nc.vector.memset(cl[c]["dist"][:], BIG))
        emit("dve", nc.vector.memset(cl[c]["outrow"][:], 0.0))
    wait("act", "gp", dma0)
    wait("pe", "gp", dma0)

    ticks = [dict() for _ in range(n_clouds)]

    def upd_head(c, s):
        t, tk = cl[c], ticks[c]
        label[0] = f"{'AB'[c]}.upd"
        for j, sq in enumerate(("sqx", "sqy", "sqz")):
            tk[sq] = emit("act", nc.scalar.activation(
                t[sq][:], t["mega"][:, j * 128:(j + 1) * 128], AT.Square,
                bias=t["biassb"][:, j:j + 1], scale=1.0))

    def upd_dve_a(c):
        t, tk = cl[c], ticks[c]
        label[0] = f"{'AB'[c]}.upd"
        wait("dve", "act", tk["sqy"])
        tk["t1"] = emit("dve", nc.vector.tensor_tensor(t["t1"][:], t["sqx"][:], t["sqy"][:], AL.add))
        wait("dve", "act", tk["sqz"])
        tk["d"] = emit("dve", nc.vector.tensor_tensor(t["dd"][:], t["t1"][:], t["sqz"][:], AL.add))

    def upd_dve_b(c):
        t, tk = cl[c], ticks[c]
        label[0] = f"{'AB'[c]}.upd"
        tk["min"] = emit("dve", nc.vector.tensor_tensor(t["dist"][:], t["dist"][:], t["dd"][:], AL.min))
        tk["rowmax"] = emit("dve", nc.vector.reduce_max(t["rowmax"][:, 0:1], t["dist"][:], axis=AX.X))

    def argmax_rmt(c):
        t, tk = cl[c], ticks[c]
        label[0] = f"{'AB'[c]}.arg"
        wait("pe", "dve", tk["rowmax"])
        tk["rmt"] = emit("pe", nc.tensor.matmul(t["rmt_ps"][:], t["rowmax"][:, 0:1], ident[:], start=True, stop=True))

    def argmax_m(c):
        t, tk = cl[c], ticks[c]
        label[0] = f"{'AB'[c]}.arg"
        wait("dve", "pe", tk["rmt"])
        tk["m"] = emit("dve", nc.vector.reduce_max(t["msb"][0:1, 0:1], t["rmt_ps"][0:1, :], axis=AX.X))

    def argmax_mb(c):
        t, tk = cl[c], ticks[c]
        label[0] = f"{'AB'[c]}.arg"
        wait("pe", "dve", tk["m"])
        tk["mb"] = emit("pe", nc.tensor.matmul(t["mb_ps"][:], onesrow[0:1, :], t["msb"][0:1, 0:1], start=True, stop=True))

    def argmax_mask(c):
        t, tk = cl[c], ticks[c]
        label[0] = f"{'AB'[c]}.arg"
        wait("dve", "pe", tk["mb"])
        tk["mask"] = emit("dve", nc.vector.tensor_tensor(
            t["mask"][:], t["dist"][:], t["mb_ps"][:, 0:1].broadcast_to((128, 128)), AL.is_equal))

    def phase_gather(c, s):
        t, tk = cl[c], ticks[c]
        label[0] = f"{'AB'[c]}.gat"
        mask_rep = t["mask"][:].rearrange("p (a f) -> p a f", a=1).broadcast_to((128, 4, 128))
        tk["mul"] = emit("dve", nc.vector.tensor_tensor(t["scr4"][:], t["mega"][:], mask_rep, AL.mult))
        tk["rs"] = emit("dve", nc.vector.tensor_reduce(
            t["rs"][:, 0:4], t["scr4"][:].rearrange("p (k f) -> p k f", k=4), axis=AX.X, op=AL.add))

    def tail_pe(c, s):
        t, tk = cl[c], ticks[c]
        label[0] = f"{'AB'[c]}.tai"
        wait("pe", "dve", tk["rs"])
        tk["negbc"] = emit("pe", nc.tensor.matmul(t["negbc_ps"][:], negones[:], t["rs"][:, 0:4], start=True, stop=True))

    def tail_act(c, s):
        t, tk = cl[c], ticks[c]
        label[0] = f"{'AB'[c]}.tai"
        wait("act", "pe", tk["negbc"])
        tk["bias"] = emit("act", nc.scalar.copy(t["biassb"][:], t["negbc_ps"][:, 0:3]))
        tk["out"] = emit("act", nc.scalar.copy(t["outrow"][0:1, 4 * s:4 * s + 4], t["negbc_ps"][0:1, 0:4]))

    def phase_update(c, s):
        upd_head(c, s)
        upd_dve_a(c)
        upd_dve_b(c)

    def phase_argmax(c, s):
        argmax_rmt(c)
        argmax_m(c)
        argmax_mb(c)
        argmax_mask(c)

    def phase_tail(c, s):
        tail_pe(c, s)
        tail_act(c, s)

    if n_clouds == 2:
        # software-pipelined at op granularity: cloud B runs ~half a step
        # behind cloud A; B's update DVE ops are slotted into A's
        # transpose/broadcast PSUM round-trip gaps.
        A, Bc = 0, 1

        def steady(s, first=False):
            upd_head(A, s)
            if not first:
                tail_pe(Bc, s - 1)
            upd_dve_a(A)
            upd_dve_b(A)
            if not first:
                tail_act(Bc, s - 1)
            argmax_rmt(A)
            argmax_m(A)
            upd_head(Bc, s)
            upd_dve_a(Bc)
            argmax_mb(A)
            argmax_mask(A)
            upd_dve_b(Bc)
            phase_gather(A, s)
            argmax_rmt(Bc)
            argmax_m(Bc)
            tail_pe(A, s)
            argmax_mb(Bc)
            tail_act(A, s)
            argmax_mask(Bc)
            phase_gather(Bc, s)

        steady(1, first=True)
        for s in range(2, S):
            steady(s)
        tail_pe(Bc, S - 1)
        tail_act(Bc, S - 1)
    else:
        for s in range(1, S):
            for c in range(n_clouds):
                phase_update(c, s)
            for c in range(n_clouds):
                phase_argmax(c, s)
            for c in range(n_clouds):
                phase_gather(c, s)
            for c in range(n_clouds):
                phase_tail(c, s)

    for c in range(n_clouds):
        wait("gp", "act", ticks[c]["out"])
        emit("gp", nc.gpsimd.dma_start(out_d[c], cl[c]["outrow"][0:1, :]), 16)

    es.close()
    return nc


def _make_inputs(pos_pair):
    ncl = pos_pair.shape[0]
    mega = np.empty((ncl, 128, 512), np.float32)
    bias0 = np.empty((ncl, 128, 3), np.float32)
    gi = np.arange(N, dtype=np.float32).reshape(128, 128)
    for c in range(ncl):
        for j in range(3):
            mega[c, :, j * 128:(j + 1) * 128] = pos_pair[c, :, j].reshape(128, 128)
        mega[c, :, 384:512] = gi
        bias0[c] = -pos_pair[c, 0]
    return {
        "mega": mega,
        "bias0": bias0,
        "ident": np.eye(128, dtype=np.float32),
        "onesrow": np.ones((1, 128), np.float32),
        "negones": np.full((128, 128), -1.0, np.float32),
    }


def _get_nc():
    if "nc" not in _CACHE:
        _CACHE["nc"] = _build_fps_kernel()
    return _CACHE["nc"]


def run_on_cores(pos, **spmd_kwargs):
    """pos [16, 16384, 3] f32 -> (idx [16*2048] int32, BassKernelResults)."""
    pos = np.ascontiguousarray(np.asarray(pos, dtype=np.float32))
    assert pos.shape == (B, N, 3)
    nc = _get_nc()
    in_maps = [_make_inputs(pos[N_CLOUDS * c: N_CLOUDS * (c + 1)]) for c in range(N_CORES)]
    res = run_bass_kernel_spmd(nc, in_maps, core_ids=list(range(N_CORES)), **spmd_kwargs)
    idx = np.empty((B, S), np.int32)
    for core in range(N_CORES):
        outrow = res.results[core]["outrow"]  # [n_clouds, 4S]
        loc = np.rint(-outrow[:, 3::4]).astype(np.int32)
        loc[:, 0] = 0
        for c in range(N_CLOUDS):
            b = N_CLOUDS * core + c
            idx[b] = loc[c] + b * N
    return idx.reshape(-1), res


def kernel(pos):
    idx, _ = run_on_cores(pos)
    return idx


# revision 17
# speedup vs baseline: 1.2798x; 1.0265x over previous
"""Farthest Point Sampling (FPS) Bass/TRN2 kernel.

Problem: pos [16, 16384, 3] f32 -> indices [16*2048] int32 (exact FPS,
start index 0, ratio 1/8), bit-exact trajectory vs the f32 reference.

Sharding: batch 16 clouds -> 8 NeuronCores, 2 clouds per core (data
parallel, no cross-core communication). Each cloud is laid out as
[128 partitions, 128 free] (point n -> (n//128, n%128)).

Per FPS step per cloud (raw bass, manual semaphores):
  ACT : SQX/SQY/SQZ = Square(coord + bias)       bias = -c (per-partition AP)
  DVE : t1 = SQX+SQY; d = t1+SQZ; DIST = min(DIST, d); rowmax = max_f(DIST)
  PE  : rmT = rowmax^T                           (matmul vs identity -> PSUM)
  DVE : M = max(rmT)                             [1,1]
  PE  : Mb = ones_row^T @ M                      broadcast M -> [128,1] PSUM
  DVE : MASK = is_equal(DIST, Mb)                single-hot (no ties, verified)
  DVE : SCR4 = MEGA4 * MASK(x4);  RS = reduce_add -> [128,4]  (MEGA4 = [X|Y|Z|GIOTA])
  PE  : NEGBC = (-1)^T128 @ RS -> PSUM [128,4]   = (-cx,-cy,-cz,-n*) everywhere
  ACT : BIAS = NEGBC[:,0:3] -> SBUF;  OUTROW[0, 4s:4s+4] = NEGBC[0,:]
Host decodes n* = -OUTROW[4s+3]."""

import numpy as np
from contextlib import ExitStack

import concourse.bass as bass
import concourse.mybir as mybir
from concourse.bass_utils import run_bass_kernel_spmd

AT = mybir.ActivationFunctionType
AL = mybir.AluOpType
AX = mybir.AxisListType
F32 = mybir.dt.float32

B, N, S = 16, 16384, 2048
N_CORES = 8
N_CLOUDS = 2  # per core
BIG = 1.0e10

_CACHE = {}
LABELS = {}


def _build_fps_kernel(S=S, n_clouds=N_CLOUDS):
    nc = bass.Bass(trn_type="TRN2", detect_race_conditions=False)
    mega_d = nc.dram_tensor("mega", [n_clouds, 128, 512], F32, kind="ExternalInput")
    bias0_d = nc.dram_tensor("bias0", [n_clouds, 128, 3], F32, kind="ExternalInput")
    ident_d = nc.dram_tensor("ident", [128, 128], F32, kind="ExternalInput")
    onesrow_d = nc.dram_tensor("onesrow", [1, 128], F32, kind="ExternalInput")
    negones_d = nc.dram_tensor("negones", [128, 128], F32, kind="ExternalInput")
    out_d = nc.dram_tensor("outrow", [n_clouds, 4 * S], F32, kind="ExternalOutput")

    es = ExitStack()
    counter = [0]

    def sb(shape):
        counter[0] += 1
        return es.enter_context(nc.sbuf_tensor(f"sb{counter[0]}", shape, F32))

    def ps(shape):
        counter[0] += 1
        return es.enter_context(nc.psum_tensor(f"ps{counter[0]}", shape, F32))

    ident = sb([128, 128])
    onesrow = sb([1, 128])
    negones = sb([128, 128])

    cl = []
    for c in range(n_clouds):
        cl.append(dict(
            mega=sb([128, 512]),
            dist=sb([128, 128]),
            sqx=sb([128, 128]), sqy=sb([128, 128]), sqz=sb([128, 128]),
            t1=sb([128, 128]), dd=sb([128, 128]),
            mask=sb([128, 128]),
            scr4=sb([128, 512]),
            rs=sb([128, 4]),
            rowmax=sb([128, 1]),
            msb=sb([1, 1]),
            biassb=sb([128, 3]),
            outrow=sb([1, 4 * S]),
            rmt_ps=ps([1, 128]),
            mb_ps=ps([128, 1]),
            negbc_ps=ps([128, 4]),
        ))

    sem_act = es.enter_context(nc.semaphore())
    sem_dve = es.enter_context(nc.semaphore())
    sem_pe = es.enter_context(nc.semaphore())
    sem_gp = es.enter_context(nc.semaphore())

    sems = {"act": sem_act, "dve": sem_dve, "pe": sem_pe, "gp": sem_gp}
    engines = {"act": nc.scalar, "dve": nc.vector, "pe": nc.tensor, "gp": nc.gpsimd}
    count = {k: 0 for k in sems}
    waited = {(a, b): 0 for a in sems for b in sems}
    label = [None]

    def emit(eng, instr, inc=1):
        instr.then_inc(sems[eng], inc)
        count[eng] += inc
        if label[0] is not None:
            try:
                LABELS[instr.ins.name] = label[0]
            except Exception:
                pass
        return count[eng]

    def wait(consumer, producer, tick):
        if tick is None or consumer == producer:
            return
        if waited[(consumer, producer)] < tick:
            engines[consumer].wait_ge(sems[producer], tick)
            waited[(consumer, producer)] = tick

    for c in range(n_clouds):
        emit("gp", nc.gpsimd.dma_start(cl[c]["mega"][:], mega_d[c]), 16)
        emit("gp", nc.gpsimd.dma_start(cl[c]["biassb"][:], bias0_d[c]), 16)
    emit("gp", nc.gpsimd.dma_start(ident[:], ident_d[:]), 16)
    emit("gp", nc.gpsimd.dma_start(onesrow[:], onesrow_d[:]), 16)
    emit("gp", nc.gpsimd.dma_start(negones[:], negones_d[:]), 16)
    dma0 = count["gp"]
    for c in range(n_clouds):
        wait("dve", "gp", dma0)
        emit("dve", nc.vector.memset(cl[c]["dist"][:], BIG))
        emit("dve", nc.vector.memset(cl[c]["outrow"][:], 0.0))
    wait("act", "gp", dma0)
    wait("pe", "gp", dma0)

    ticks = [dict() for _ in range(n_clouds)]

    def upd_head(c, s):
        t, tk = cl[c], ticks[c]
        label[0] = f"{'AB'[c]}.upd"
        for j, sq in enumerate(("sqx", "sqy", "sqz")):
            tk[sq] = emit("act", nc.scalar.activation(
                t[sq][:], t["mega"][:, j * 128:(j + 1) * 128], AT.Square,
                bias=t["biassb"][:, j:j + 1], scale=1.0))

    def upd_dve_a(c):
        t, tk = cl[c], ticks[c]
        label[0] = f"{'AB'[c]}.upd"
        wait("dve", "act", tk["sqy"])
        tk["t1"] = emit("dve", nc.vector.tensor_tensor(t["t1"][:], t["sqx"][:], t["sqy"][:], AL.add))
        wait("dve", "act", tk["sqz"])
        tk["d"] = emit("dve", nc.vector.tensor_tensor(t["dd"][:], t["t1"][:], t["sqz"][:], AL.add))

    def upd_dve_b(c):
        t, tk = cl[c], ticks[c]
        label[0] = f"{'AB'[c]}.upd"
        tk["min"] = emit("dve", nc.vector.tensor_tensor(t["dist"][:], t["dist"][:], t["dd"][:], AL.min))
        tk["rowmax"] = emit("dve", nc.vector.reduce_max(t["rowmax"][:, 0:1], t["dist"][:], axis=AX.X))

    def argmax_rmt(c):
        t, tk = cl[c], ticks[c]
        label[0] = f"{'AB'[c]}.arg"
        wait("pe", "dve", tk["rowmax"])
        tk["rmt"] = emit("pe", nc.tensor.matmul(t["rmt_ps"][:], t["rowmax"][:, 0:1], ident[:], start=True, stop=True))

    def argmax_m(c):
        t, tk = cl[c], ticks[c]
        label[0] = f"{'AB'[c]}.arg"
        wait("dve", "pe", tk["rmt"])
        tk["m"] = emit("dve", nc.vector.reduce_max(t["msb"][0:1, 0:1], t["rmt_ps"][0:1, :], axis=AX.X))

    def argmax_mb(c):
        t, tk = cl[c], ticks[c]
        label[0] = f"{'AB'[c]}.arg"
        wait("pe", "dve", tk["m"])
        tk["mb"] = emit("pe", nc.tensor.matmul(t["mb_ps"][:], onesrow[0:1, :], t["msb"][0:1, 0:1], start=True, stop=True))

    def argmax_mask(c):
        t, tk = cl[c], ticks[c]
        label[0] = f"{'AB'[c]}.arg"
        wait("dve", "pe", tk["mb"])
        tk["mask"] = emit("dve", nc.vector.tensor_tensor(
            t["mask"][:], t["dist"][:], t["mb_ps"][:, 0:1].broadcast_to((128, 128)), AL.is_equal))

    def gather_mul(c):
        t, tk = cl[c], ticks[c]
        label[0] = f"{'AB'[c]}.gat"
        mask_rep = t["mask"][:].rearrange("p (a f) -> p a f", a=1).broadcast_to((128, 4, 128))
        tk["mul"] = emit("dve", nc.vector.tensor_tensor(t["scr4"][:], t["mega"][:], mask_rep, AL.mult))

    def gather_red(c):
        t, tk = cl[c], ticks[c]
        label[0] = f"{'AB'[c]}.gat"
        tk["rs"] = emit("dve", nc.vector.tensor_reduce(
            t["rs"][:, 0:4], t["scr4"][:].rearrange("p (k f) -> p k f", k=4), axis=AX.X, op=AL.add))

    def phase_gather(c, s):
        gather_mul(c)
        gather_red(c)

    def tail_pe(c, s):
        t, tk = cl[c], ticks[c]
        label[0] = f"{'AB'[c]}.tai"
        wait("pe", "dve", tk["rs"])
        tk["negbc"] = emit("pe", nc.tensor.matmul(t["negbc_ps"][:], negones[:], t["rs"][:, 0:4], start=True, stop=True))

    def tail_act(c, s):
        t, tk = cl[c], ticks[c]
        label[0] = f"{'AB'[c]}.tai"
        wait("act", "pe", tk["negbc"])
        tk["bias"] = emit("act", nc.scalar.copy(t["biassb"][:], t["negbc_ps"][:, 0:3]))
        tk["out"] = emit("act", nc.scalar.copy(t["outrow"][0:1, 4 * s:4 * s + 4], t["negbc_ps"][0:1, 0:4]))

    def phase_update(c, s):
        upd_head(c, s)
        upd_dve_a(c)
        upd_dve_b(c)

    def phase_argmax(c, s):
        argmax_rmt(c)
        argmax_m(c)
        argmax_mb(c)
        argmax_mask(c)

    def phase_tail(c, s):
        tail_pe(c, s)
        tail_act(c, s)

    if n_clouds == 2:
        # software-pipelined at op granularity: cloud B runs ~half a step
        # behind cloud A; B's update DVE ops are slotted into A's
        # transpose/broadcast PSUM round-trip gaps.
        A, Bc = 0, 1

        def steady(s, first=False):
            upd_head(A, s)
            if not first:
                tail_pe(Bc, s - 1)
            upd_dve_a(A)
            upd_dve_b(A)
            if not first:
                tail_act(Bc, s - 1)
            argmax_rmt(A)
            argmax_m(A)
            upd_head(Bc, s)
            upd_dve_a(Bc)
            argmax_mb(A)
            argmax_mask(A)
            upd_dve_b(Bc)
            gather_mul(A)
            argmax_rmt(Bc)
            argmax_m(Bc)
            gather_red(A)
            argmax_mb(Bc)
            tail_pe(A, s)
            tail_act(A, s)
            argmax_mask(Bc)
            phase_gather(Bc, s)

        steady(1, first=True)
        for s in range(2, S):
            steady(s)
        tail_pe(Bc, S - 1)
        tail_act(Bc, S - 1)
    else:
        for s in range(1, S):
            for c in range(n_clouds):
                phase_update(c, s)
            for c in range(n_clouds):
                phase_argmax(c, s)
            for c in range(n_clouds):
                phase_gather(c, s)
            for c in range(n_clouds):
                phase_tail(c, s)

    for c in range(n_clouds):
        wait("gp", "act", ticks[c]["out"])
        emit("gp", nc.gpsimd.dma_start(out_d[c], cl[c]["outrow"][0:1, :]), 16)

    es.close()
    return nc


def _make_inputs(pos_pair):
    ncl = pos_pair.shape[0]
    mega = np.empty((ncl, 128, 512), np.float32)
    bias0 = np.empty((ncl, 128, 3), np.float32)
    gi = np.arange(N, dtype=np.float32).reshape(128, 128)
    for c in range(ncl):
        for j in range(3):
            mega[c, :, j * 128:(j + 1) * 128] = pos_pair[c, :, j].reshape(128, 128)
        mega[c, :, 384:512] = gi
        bias0[c] = -pos_pair[c, 0]
    return {
        "mega": mega,
        "bias0": bias0,
        "ident": np.eye(128, dtype=np.float32),
        "onesrow": np.ones((1, 128), np.float32),
        "negones": np.full((128, 128), -1.0, np.float32),
    }


def _get_nc():
    if "nc" not in _CACHE:
        _CACHE["nc"] = _build_fps_kernel()
    return _CACHE["nc"]


def run_on_cores(pos, **spmd_kwargs):
    """pos [16, 16384, 3] f32 -> (idx [16*2048] int32, BassKernelResults)."""
    pos = np.ascontiguousarray(np.asarray(pos, dtype=np.float32))
    assert pos.shape == (B, N, 3)
    nc = _get_nc()
    in_maps = [_make_inputs(pos[N_CLOUDS * c: N_CLOUDS * (c + 1)]) for c in range(N_CORES)]
    res = run_bass_kernel_spmd(nc, in_maps, core_ids=list(range(N_CORES)), **spmd_kwargs)
    idx = np.empty((B, S), np.int32)
    for core in range(N_CORES):
        outrow = res.results[core]["outrow"]  # [n_clouds, 4S]
        loc = np.rint(-outrow[:, 3::4]).astype(np.int32)
        loc[:, 0] = 0
        for c in range(N_CLOUDS):
            b = N_CLOUDS * core + c
            idx[b] = loc[c] + b * N
    return idx.reshape(-1), res


def kernel(pos):
    idx, _ = run_on_cores(pos)
    return idx
